# revision 35
# baseline (speedup 1.0000x reference)
"""BiasedSelfAttention Trainium2 kernel, 8-core SPMD — transposed fp8 scheme.

Reference computation (per batch b, head h):
    qkv = x @ W_attn + b_attn;  Q,K,V = split(qkv)
    S   = Q K^T / sqrt(hd)
    A   = softmax(S, axis=-1) + attn_B          (post-softmax additive bias)
    y   = A @ V

Sharding: 2 batches x 16 heads = 32 (b,h) pairs -> 4 heads/core,
core c handles batch c//4, heads [4*(c%4), 4*(c%4)+4).

Per-core kernel (TRANSPOSED orientation — no A assembly, no A^T transposes):
  - QKV projection in bf16 (Q^T/K^T per head-pair packed on partitions)
  - S^T = K Q^T computed directly on PE (bf16 in, f32 PSUM)
  - exp(S^T/8 - 3) on ACT -> expST fp8e4 (shift keeps max ~e^2.8 << 240;
    softmax shift-invariance makes it exact since denominators use the
    same quantized values)
  - U'^T = V'^T expST via fp8 DoubleRow matmuls (2 sk-chunks/instr, 0.5
    cyc/row): V' = [8*V, 8] ones-augmented -> row 64 = 8*softmax-denoms FREE.
    fp8 is safe here: the whole softmax term is ~25x smaller than the BV term.
  - BV^T = V^T B^T in bf16 (fp8 would put ~6% on the DOMINANT y component:
    error and signal both grow as sqrt(N) in the sum, no averaging-down)
  - y chunk = transpose(U'^T)*recip(denom) + transpose(BV^T); y stored bf16,
    cast to f32 on host
  - B^T host-prepped fp8 in DMA-friendly layout (8KB contiguous per
    partition -> 128-descriptor DMAs)
  - software pipelining at tile granularity: S^T(h) chunk production
    interleaved with U/BV(h-1) work units in the PE FIFO; S^T(0)
    interleaved with the QKV projection itself.
"""

import numpy as np
import ml_dtypes


def _to_bf16(a):
    return a.astype(ml_dtypes.bfloat16)


def _to_fp8(a):
    return a.astype(ml_dtypes.float8_e4m3)


import concourse.bass as bass
import concourse.mybir as mybir
import concourse.tile as tile
from concourse import bacc
from concourse.bass_utils import run_bass_kernel_spmd
from concourse.masks import make_identity

B, S, D = 2, 2048, 1024
H, HD = 16, 64
NCORES = 8
HPC = 4                 # heads per core
GD = HPC * HD           # 256 per-core output columns
KO = D // 128           # 8 contraction chunks for QKV
SQ = S // 128           # 16 seq chunks of 128
ST = S // 512           # 4 seq tiles of 512
KC2 = SQ // 2           # 8 double-chunks for fp8 DoubleRow

fp32 = mybir.dt.float32
fp32r = mybir.dt.float32r
bf16 = mybir.dt.bfloat16
fp8 = mybir.dt.float8e4

EXP_SHIFT = -3.0        # exp(S/8 - 3): keeps fp8 range safe; cancels in softmax
BSCALE = 64.0           # host-side scale on B^T for fp8 resolution
VSCALE = 8.0            # device-side scale on V for fp8 resolution

_CACHED_NC = None


def build_nc(repeat=1):
    """repeat>1 wraps the whole body in a hardware loop (for HW timing)."""
    nc = bacc.Bacc()

    xT = nc.declare_dram_parameter("xT", [D, S], bf16, isOutput=False)
    # all QKV weights packed per-partition-contiguous:
    # wall[p, ko, i*256+qk*128+m] = W col m of head-pair i Q/K (d = ko*128+p),
    # wall[p, ko, 512+n] = V weight col n
    wall = nc.declare_dram_parameter("wall", [128, KO, 768], bf16,
                                     isOutput=False)
    bqk = nc.declare_dram_parameter("bqk", [128, 2, 2], fp32, isOutput=False)
    bv = nc.declare_dram_parameter("bv", [1, GD], bf16, isOutput=False)
    # host-prepped transposed bias, bf16, quarter-major:
    # BT2[h, q, p, kc, c] = attn_B[bi, h0+h, q*512+c, kc*128+p]
    BT2 = nc.declare_dram_parameter("BT2", [HPC, 4, 128, SQ, 512], bf16,
                                    isOutput=False)
    y = nc.declare_dram_parameter("y", [S, GD], bf16, isOutput=True)

    import contextlib

    DR = mybir.MatmulPerfMode.DoubleRow

    with tile.TileContext(nc) as tc:
        with (
            tc.For_i(0, repeat, 1) if repeat > 1 else contextlib.nullcontext(),
            tc.tile_pool(name="persist", bufs=1) as persist,
            tc.tile_pool(name="small", bufs=1) as small,
        ):
            # ---- persistent SBUF tensors ----
            # per head-pair: partitions 0:64 = head 2i, 64:128 = head 2i+1;
            # free dim: [:, 0, :] = Q^T rows, [:, 1, :] = K^T rows
            qk2 = [persist.tile([128, 2, S], bf16, tag=f"qk2_{i}", name=f"qk2_{i}")
                   for i in range(HPC // 2)]
            # V' for DoubleRow: [p, kc2, h, j, 0:64] = VSCALE*V row
            # (sk = kc2*256 + j*128 + p, head h); [..., 64] = VSCALE (ones col
            # -> free softmax denominators). Inner dim padded to 80 so the
            # j-step (80 fp8 bytes) is 16B-aligned for DoubleRow ldweights.
            v65 = persist.tile([128, KC2, HPC, 2, 80], fp8, tag="v65")
            # bf16 V (unscaled, no ones col) for the BV^T matmuls
            v64b = persist.tile([128, SQ, HPC, 64], bf16, tag="v64b")
            y_sb = persist.tile([128, SQ, GD], bf16, tag="y_sb")

            ident_f32 = small.tile([128, 128], fp32, tag="ident_f32")
            make_identity(nc, ident_f32)
            bqk_sb = small.tile([128, 2, 2], fp32, tag="bqk_sb")
            nc.sync.dma_start(out=bqk_sb, in_=bqk[:, :])
            bv_sb = small.tile([1, GD], bf16, tag="bv_sb")
            nc.sync.dma_start(out=bv_sb, in_=bv[:, :])
            ones1 = small.tile([1, 128], bf16, tag="ones1")
            nc.vector.memset(ones1, 1.0)
            nc.vector.memset(v65[:, :, :, :, 64:65], VSCALE)
            eshift = small.tile([128, 1], fp32, tag="eshift")
            nc.vector.memset(eshift, EXP_SHIFT)

            yv = y.rearrange("(c p) n -> p c n", p=128)

            with (
                # phase2a pools: outlive phase 1 (stack allocator is LIFO)
                tc.tile_pool(name="expstp", bufs=2) as expstp,
                tc.tile_pool(name="pspool", bufs=2, space="PSUM") as pspool,
            ):
                def s_pair_chunk(pair, e0, e1, kc):
                    """S^T chunk kc for BOTH heads of a pair, matmuls
                    interleaved between row groups (0,0)/(64,0) so they run
                    concurrently in the PE array (K=64 each)."""
                    for t in range(2):
                        ps0 = pspool.tile([128, 1024], fp32, tag="ps",
                                          name="ps0")
                        ps1 = pspool.tile([128, 1024], fp32, tag="ps",
                                          name="ps1")
                        for t2 in range(2):
                            c0 = t * 1024 + t2 * 512
                            for j, ps in ((0, ps0), (1, ps1)):
                                off = 64 * j
                                nc.tensor.matmul(
                                    ps[:, t2 * 512:(t2 + 1) * 512],
                                    qk2[pair][off:off + 64, 1,
                                              kc * 128:(kc + 1) * 128],
                                    qk2[pair][off:off + 64, 0, c0:c0 + 512],
                                    start=True,
                                    stop=True,
                                )
                        for e, ps in ((e0, ps0), (e1, ps1)):
                            nc.scalar.activation(
                                e[:, kc, t * 1024:(t + 1) * 1024],
                                ps,
                                mybir.ActivationFunctionType.Exp,
                                bias=eshift[:, 0:1],
                                scale=0.125,
                            )

                def emit_spair_interleaved(pair, e0, e1, fill):
                    """Emit all S^T chunks of a head pair, draining fill units
                    between chunks to keep the PE FIFO busy."""
                    n = len(fill)
                    done = 0
                    for kc in range(SQ):
                        s_pair_chunk(pair, e0, e1, kc)
                        upto = (kc + 1) * n // SQ
                        while done < upto:
                            fill[done]()
                            done += 1

                # ---- phase 1 (interleaved with S^T of head 0) ----
                with (
                    tc.tile_pool(name="p1sb", bufs=1) as p1sb,
                    tc.tile_pool(name="p1ps", bufs=2, space="PSUM") as p1ps,
                    tc.tile_pool(name="p1psv", bufs=2, space="PSUM") as p1psv,
                ):
                    wall_sb = p1sb.tile([128, KO, 768], bf16, tag="wall_sb")
                    nc.scalar.dma_start(out=wall_sb, in_=wall[:, :])
                    xts = p1sb.tile([128, KO, S], bf16, tag="xts")
                    xv = xT.rearrange("(ko p) s -> p ko s", p=128)
                    for ko in range(KO):
                        eng = nc.sync if ko % 2 == 0 else nc.scalar
                        eng.dma_start(out=xts[:, ko, :], in_=xv[:, ko, :])

                    def qk_tile(i, qk, t):
                        ps = p1ps.tile([128, 512], fp32, tag="ps_qk",
                                       name="ps_qk")
                        w0 = i * 256 + qk * 128
                        for ko in range(KO):
                            nc.tensor.matmul(
                                ps,
                                wall_sb[:, ko, w0:w0 + 128],
                                xts[:, ko, t * 512:(t + 1) * 512],
                                start=(ko == 0),
                                stop=(ko == KO - 1),
                            )
                        # PSUM -> SBUF + per-partition bias
                        nc.scalar.activation(
                            qk2[i][:, qk, t * 512:(t + 1) * 512],
                            ps,
                            mybir.ActivationFunctionType.Identity,
                            bias=bqk_sb[:, i, qk:qk + 1],
                            scale=1.0,
                        )

                    def v_unit(kc):
                        psv = p1psv.tile([128, GD], fp32, tag="ps_v",
                                         name="ps_v")
                        for ko in range(KO):
                            nc.tensor.matmul(
                                psv,
                                xts[:, ko, kc * 128:(kc + 1) * 128],
                                wall_sb[:, ko, 512:768],
                                start=(ko == 0),
                                stop=False,
                            )
                        nc.tensor.matmul(psv, ones1, bv_sb, start=False,
                                         stop=True)
                        for h in range(HPC):
                            # VSCALE*V, cast to fp8, DoubleRow interleave slot
                            nc.vector.tensor_scalar_mul(
                                v65[:, kc // 2, h, kc % 2, 0:64],
                                psv[:, h * HD:(h + 1) * HD],
                                VSCALE,
                            )
                            nc.vector.tensor_copy(
                                v64b[:, kc, h, :], psv[:, h * HD:(h + 1) * HD]
                            )

                    # heads 0/1 need pair-0 QK: emit pair-0 upfront,
                    # interleave pair-1 QK + V production with S^T(0,1)
                    for qk in range(2):
                        for t in range(ST):
                            qk_tile(0, qk, t)
                    fill = []
                    for qk in range(2):
                        for t in range(ST):
                            fill.append(lambda qk=qk, t=t: qk_tile(1, qk, t))
                    for kc in range(SQ):
                        fill.append(lambda kc=kc: v_unit(kc))

                    e_h = [None] * HPC
                    e_h[0] = expstp.tile([128, SQ, S], fp8, tag="expst",
                                         name="expst0")
                    e_h[1] = expstp.tile([128, SQ, S], fp8, tag="expst",
                                         name="expst1")
                    emit_spair_interleaved(0, e_h[0], e_h[1], fill)

                # ---- phase 2b: U'/BV + assembly, pipelined with S^T(h) ----
                with (
                    tc.tile_pool(name="expstp2", bufs=2) as expstp2,
                    tc.tile_pool(name="ubvpool", bufs=2, space="PSUM") as ubvpool,
                    tc.tile_pool(name="trpool", bufs=2, space="PSUM") as trpool,
                    tc.tile_pool(name="btring", bufs=3) as btring,
                    tc.tile_pool(name="asmpool", bufs=2) as asmpool,
                    tc.tile_pool(name="rcpool", bufs=4) as rcpool,
                ):
                    def ubv_units(h, e):
                        """Work units for U'^T/BV^T + y assembly of head h."""
                        units = []
                        for q in range(4):
                            def load_bt(half, h=h, q=q):
                                # half-quarter [128, 8, 512] bf16 (8 KB/part)
                                bt = btring.tile([128, 8, 512], bf16, tag="bt",
                                                 name="bt")
                                eng = nc.sync if (q + half) % 2 == 0 else nc.scalar
                                eng.dma_start(
                                    out=bt,
                                    in_=BT2[h, q][:, half * 8:half * 8 + 8, :],
                                )
                                return bt

                            # closure cell: tiles created lazily at drain time
                            cell = {}

                            def start_q(cell=cell, load_bt=load_bt):
                                cell["bt0"] = load_bt(0)
                                cell["u"] = ubvpool.tile([65, 512], fp32,
                                                         tag="ub", name="u")
                                cell["b"] = ubvpool.tile([64, 512], fp32,
                                                         tag="ub", name="bb")

                            units.append(start_q)

                            def load_bt1(cell=cell, load_bt=load_bt):
                                cell["bt1"] = load_bt(1)

                            def mm_pair(kc2, cell=cell, h=h, q=q, e=e):
                                st, sp = (kc2 == 0), (kc2 == KC2 - 1)
                                nc.tensor.matmul(
                                    cell["u"],
                                    v65[:, kc2, h, :, 0:65],
                                    e[:, 2 * kc2:2 * kc2 + 2,
                                      q * 512:(q + 1) * 512],
                                    start=st, stop=sp, perf_mode=DR,
                                )
                                bt = cell["bt0"] if kc2 < 4 else cell["bt1"]
                                for j in range(2):
                                    kc = 2 * kc2 + j
                                    nc.tensor.matmul(
                                        cell["b"],
                                        v64b[:, kc, h, :],
                                        bt[:, kc % 8, :],
                                        start=(kc == 0), stop=(kc == SQ - 1),
                                    )

                            for kc2 in range(KC2):
                                units.append(lambda kc2=kc2, f=mm_pair: f(kc2))
                                if kc2 == 0:
                                    units.append(load_bt1)

                            def copies(cell=cell):
                                usb = asmpool.tile([65, 512], fp32, tag="usb",
                                                   name="usb")
                                bsb = asmpool.tile([64, 512], fp32, tag="bsb",
                                                   name="bsb")
                                nc.vector.tensor_copy(usb, cell["u"])
                                nc.vector.tensor_copy(bsb, cell["b"])
                                cell["usb"], cell["bsb"] = usb, bsb

                            units.append(copies)

                            def asm(cc, cell=cell, h=h, q=q):
                                c = q * 4 + cc
                                tr = trpool.tile([128, 2, 65], fp32, tag="tr",
                                                 name="tr")
                                nc.tensor.transpose(
                                    tr[:, 0, :],
                                    cell["usb"][:, cc * 128:(cc + 1) * 128],
                                    ident_f32[0:65, 0:65],
                                )
                                nc.tensor.transpose(
                                    tr[:, 1, 0:64],
                                    cell["bsb"][:, cc * 128:(cc + 1) * 128],
                                    ident_f32[0:64, 0:64],
                                )
                                rc = rcpool.tile([128, 1], fp32, tag="rc",
                                                 name="rc")
                                nc.vector.reciprocal(rc, tr[:, 0, 64:65])
                                # y = U*recip + BV (two ops: only one PSUM
                                # input allowed per DVE instruction)
                                ys = y_sb[:, c, h * HD:(h + 1) * HD]
                                nc.vector.tensor_scalar_mul(
                                    ys, tr[:, 0, 0:64], rc
                                )
                                nc.vector.tensor_tensor(
                                    ys, ys, tr[:, 1, 0:64],
                                    mybir.AluOpType.add,
                                )
                                if h == HPC - 1:
                                    nc.sync.dma_start(
                                        out=yv[:, c, :], in_=y_sb[:, c, :]
                                    )

                            for cc in range(4):
                                units.append(lambda cc=cc, f=asm: f(cc))
                        return units

                    e_h[2] = expstp2.tile([128, SQ, S], fp8, tag="expst",
                                          name="expst2")
                    e_h[3] = expstp2.tile([128, SQ, S], fp8, tag="expst",
                                          name="expst3")
                    fill2 = ubv_units(0, e_h[0]) + ubv_units(1, e_h[1])
                    emit_spair_interleaved(1, e_h[2], e_h[3], fill2)
                    for unit in ubv_units(2, e_h[2]) + ubv_units(3, e_h[3]):
                        unit()

    nc.finalize()
    return nc


def _prep_core_inputs(x, attn_B, W_attn, b_attn, core, BT_all=None):
    bi, g = core // 4, core % 4
    h0 = HPC * g
    xT = _to_bf16(np.ascontiguousarray(x[bi].T))             # [D, S]
    wqk = np.empty((D, 2, 2, 128), np.float32)
    bqk = np.empty((128, 2, 2), np.float32)
    for i in range(HPC // 2):
        for j in range(2):                                   # head within pair
            gh = h0 + 2 * i + j
            sl = slice(64 * j, 64 * j + 64)
            wqk[:, i, 0, sl] = W_attn[:, gh * 64:(gh + 1) * 64]
            wqk[:, i, 1, sl] = W_attn[:, D + gh * 64:D + (gh + 1) * 64]
            bqk[sl, i, 0] = b_attn[gh * 64:(gh + 1) * 64]
            bqk[sl, i, 1] = b_attn[D + gh * 64:D + (gh + 1) * 64]
    wv = W_attn[:, 2 * D + g * GD:2 * D + (g + 1) * GD]
    bv = b_attn[2 * D + g * GD:2 * D + (g + 1) * GD]
    # pack per-partition-contiguous: wall[p, ko, :512] = wqk, [512:] = wv
    wall = np.empty((128, KO, 768), np.float32)
    wall[:, :, 0:512] = wqk.reshape(KO, 128, 512).transpose(1, 0, 2)
    wall[:, :, 512:768] = wv.reshape(KO, 128, GD).transpose(1, 0, 2)
    if BT_all is None:
        BT_all = _prep_bt(attn_B)
    BT2 = np.ascontiguousarray(BT_all[bi, h0:h0 + HPC])
    return {
        "xT": xT, "wall": _to_bf16(wall), "bqk": bqk,
        "bv": _to_bf16(bv.reshape(1, GD)), "BT2": BT2,
    }


def _prep_bt(attn_B):
    """[b, h, sq, sk] f32 -> [b, h, q, p, kc, c] bf16 where
    BT2[b, h, q, p, kc, c] = attn_B[b, h, q*512+c, kc*128+p]."""
    a = _to_bf16(attn_B)                           # [b, h, 2048(sq), 2048(sk)]
    a = a.reshape(B, H, 4, 512, SQ, 128)           # [b, h, q, c, kc, p]
    return a.transpose(0, 1, 2, 5, 4, 3)           # [b, h, q, p, kc, c]


def kernel(x, attn_B, W_attn, b_attn):
    global _CACHED_NC
    x = np.asarray(x, np.float32)
    attn_B = np.asarray(attn_B, np.float32)
    W_attn = np.asarray(W_attn, np.float32)
    b_attn = np.asarray(b_attn, np.float32)

    if _CACHED_NC is None:
        _CACHED_NC = build_nc()
    nc = _CACHED_NC

    BT_all = np.ascontiguousarray(_prep_bt(attn_B))
    in_maps = [
        _prep_core_inputs(x, attn_B, W_attn, b_attn, c, BT_all=BT_all)
        for c in range(NCORES)
    ]
    res = run_bass_kernel_spmd(nc, in_maps, core_ids=list(range(NCORES)))

    out = np.empty((B, S, D), np.float32)
    for c in range(NCORES):
        bi, g = c // 4, c % 4
        out[bi, :, g * GD:(g + 1) * GD] = res.results[c]["y"].astype(np.float32)
    return out


# revision 39
# speedup vs baseline: 1.0195x; 1.0195x over previous
"""BiasedSelfAttention Trainium2 kernel, 8-core SPMD — transposed fp8 scheme.

Reference computation (per batch b, head h):
    qkv = x @ W_attn + b_attn;  Q,K,V = split(qkv)
    S   = Q K^T / sqrt(hd)
    A   = softmax(S, axis=-1) + attn_B          (post-softmax additive bias)
    y   = A @ V

Sharding: 2 batches x 16 heads = 32 (b,h) pairs -> 4 heads/core,
core c handles batch c//4, heads [4*(c%4), 4*(c%4)+4).

Per-core kernel (TRANSPOSED orientation — no A assembly, no A^T transposes):
  - QKV projection in bf16 (Q^T/K^T per head-pair packed on partitions)
  - S^T = K Q^T computed directly on PE (bf16 in, f32 PSUM)
  - exp(S^T/8 - 3) on ACT -> expST fp8e4 (shift keeps max ~e^2.8 << 240;
    softmax shift-invariance makes it exact since denominators use the
    same quantized values)
  - U'^T = V'^T expST via fp8 DoubleRow matmuls (2 sk-chunks/instr, 0.5
    cyc/row): V' = [8*V, 8] ones-augmented -> row 64 = 8*softmax-denoms FREE.
    fp8 is safe here: the whole softmax term is ~25x smaller than the BV term.
  - BV^T = V^T B^T in bf16 (fp8 would put ~6% on the DOMINANT y component:
    error and signal both grow as sqrt(N) in the sum, no averaging-down)
  - y chunk = transpose(U'^T)*recip(denom) + transpose(BV^T); y stored bf16,
    cast to f32 on host
  - B^T host-prepped fp8 in DMA-friendly layout (8KB contiguous per
    partition -> 128-descriptor DMAs)
  - software pipelining at tile granularity: S^T(h) chunk production
    interleaved with U/BV(h-1) work units in the PE FIFO; S^T(0)
    interleaved with the QKV projection itself.
"""

import numpy as np
import ml_dtypes


def _to_bf16(a):
    return a.astype(ml_dtypes.bfloat16)


def _to_fp8(a):
    return a.astype(ml_dtypes.float8_e4m3)


import concourse.bass as bass
import concourse.mybir as mybir
import concourse.tile as tile
from concourse import bacc
from concourse.bass_utils import run_bass_kernel_spmd
from concourse.masks import make_identity

B, S, D = 2, 2048, 1024
H, HD = 16, 64
NCORES = 8
HPC = 4                 # heads per core
GD = HPC * HD           # 256 per-core output columns
KO = D // 128           # 8 contraction chunks for QKV
SQ = S // 128           # 16 seq chunks of 128
ST = S // 512           # 4 seq tiles of 512
KC2 = SQ // 2           # 8 double-chunks for fp8 DoubleRow

fp32 = mybir.dt.float32
fp32r = mybir.dt.float32r
bf16 = mybir.dt.bfloat16
fp8 = mybir.dt.float8e4

EXP_SHIFT = -3.0        # exp(S/8 - 3): keeps fp8 range safe; cancels in softmax
BSCALE = 64.0           # host-side scale on B^T for fp8 resolution
VSCALE = 8.0            # device-side scale on V for fp8 resolution

_CACHED_NC = None


def build_nc(repeat=1, skip=()):
    """repeat>1 wraps the whole body in a hardware loop (for HW timing).
    skip: ablation flags for timing attribution — any of
    {"exp", "smm", "umm", "bmm", "asm", "btdma"} (breaks correctness)."""
    skip = set(skip)
    nc = bacc.Bacc()

    xT = nc.declare_dram_parameter("xT", [D, S], bf16, isOutput=False)
    # all QKV weights packed per-partition-contiguous:
    # wall[p, ko, i*256+qk*128+m] = W col m of head-pair i Q/K (d = ko*128+p),
    # wall[p, ko, 512+n] = V weight col n
    wall = nc.declare_dram_parameter("wall", [128, KO, 768], bf16,
                                     isOutput=False)
    bqk = nc.declare_dram_parameter("bqk", [128, 2, 2], fp32, isOutput=False)
    bv = nc.declare_dram_parameter("bv", [1, GD], bf16, isOutput=False)
    # host-prepped transposed bias, bf16, quarter-major:
    # BT2[h, q, p, kc, c] = attn_B[bi, h0+h, q*512+c, kc*128+p]
    BT2 = nc.declare_dram_parameter("BT2", [HPC, 4, 128, SQ, 512], bf16,
                                    isOutput=False)
    # raw U'^T / BV^T quarter outputs; final normalize+transpose on host
    UO = nc.declare_dram_parameter("UO", [HPC, 4, 65, 512], bf16, isOutput=True)
    BO = nc.declare_dram_parameter("BO", [HPC, 4, 64, 512], bf16, isOutput=True)

    import contextlib

    DR = mybir.MatmulPerfMode.DoubleRow

    with tile.TileContext(nc) as tc:
        with (
            tc.For_i(0, repeat, 1) if repeat > 1 else contextlib.nullcontext(),
            tc.tile_pool(name="persist", bufs=1) as persist,
            tc.tile_pool(name="small", bufs=1) as small,
        ):
            # ---- persistent SBUF tensors ----
            # per head-pair: partitions 0:64 = head 2i, 64:128 = head 2i+1;
            # free dim: [:, 0, :] = Q^T rows, [:, 1, :] = K^T rows
            qk2 = [persist.tile([128, 2, S], bf16, tag=f"qk2_{i}", name=f"qk2_{i}")
                   for i in range(HPC // 2)]
            # V' for DoubleRow: [p, kc2, h, j, 0:64] = VSCALE*V row
            # (sk = kc2*256 + j*128 + p, head h); [..., 64] = VSCALE (ones col
            # -> free softmax denominators). Inner dim padded to 80 so the
            # j-step (80 fp8 bytes) is 16B-aligned for DoubleRow ldweights.
            v65 = persist.tile([128, KC2, HPC, 2, 80], fp8, tag="v65")
            # bf16 V (unscaled, no ones col) for the BV^T matmuls
            v64b = persist.tile([128, SQ, HPC, 64], bf16, tag="v64b")

            bqk_sb = small.tile([128, 2, 2], fp32, tag="bqk_sb")
            nc.sync.dma_start(out=bqk_sb, in_=bqk[:, :])
            bv_sb = small.tile([1, GD], bf16, tag="bv_sb")
            nc.sync.dma_start(out=bv_sb, in_=bv[:, :])
            ones1 = small.tile([1, 128], bf16, tag="ones1")
            nc.vector.memset(ones1, 1.0)
            nc.vector.memset(v65[:, :, :, :, 64:65], VSCALE)
            eshift = small.tile([128, 1], fp32, tag="eshift")
            nc.vector.memset(eshift, EXP_SHIFT)

            with (
                # phase2a pools: outlive phase 1 (stack allocator is LIFO)
                tc.tile_pool(name="expstp", bufs=2) as expstp,
                tc.tile_pool(name="pspool", bufs=2, space="PSUM") as pspool,
            ):
                def s_pair_chunk(pair, e0, e1, kc):
                    """S^T chunk kc for BOTH heads of a pair, matmuls
                    interleaved between row groups (0,0)/(64,0) so they run
                    concurrently in the PE array (K=64 each)."""
                    for t in range(2):
                        ps0 = pspool.tile([128, 1024], fp32, tag="ps",
                                          name="ps0")
                        ps1 = pspool.tile([128, 1024], fp32, tag="ps",
                                          name="ps1")
                        for t2 in range(2):
                            if "smm" in skip:
                                break
                            c0 = t * 1024 + t2 * 512
                            for j, ps in ((0, ps0), (1, ps1)):
                                off = 64 * j
                                nc.tensor.matmul(
                                    ps[:, t2 * 512:(t2 + 1) * 512],
                                    qk2[pair][off:off + 64, 1,
                                              kc * 128:(kc + 1) * 128],
                                    qk2[pair][off:off + 64, 0, c0:c0 + 512],
                                    start=True,
                                    stop=True,
                                )
                        for e, ps in ((e0, ps0), (e1, ps1)):
                            if "exp" in skip:
                                break
                            nc.scalar.activation(
                                e[:, kc, t * 1024:(t + 1) * 1024],
                                ps,
                                mybir.ActivationFunctionType.Exp,
                                bias=eshift[:, 0:1],
                                scale=0.125,
                            )

                def emit_spair_interleaved(pair, e0, e1, fill):
                    """Emit all S^T chunks of a head pair, draining fill units
                    between chunks to keep the PE FIFO busy."""
                    n = len(fill)
                    done = 0
                    for kc in range(SQ):
                        s_pair_chunk(pair, e0, e1, kc)
                        upto = (kc + 1) * n // SQ
                        while done < upto:
                            fill[done]()
                            done += 1

                # ---- phase 1 (interleaved with S^T of head 0) ----
                with (
                    tc.tile_pool(name="p1sb", bufs=1) as p1sb,
                    tc.tile_pool(name="p1ps", bufs=2, space="PSUM") as p1ps,
                    tc.tile_pool(name="p1psv", bufs=2, space="PSUM") as p1psv,
                ):
                    wall_sb = p1sb.tile([128, KO, 768], bf16, tag="wall_sb")
                    nc.scalar.dma_start(out=wall_sb, in_=wall[:, :])
                    xts = p1sb.tile([128, KO, S], bf16, tag="xts")
                    xv = xT.rearrange("(ko p) s -> p ko s", p=128)
                    for ko in range(KO):
                        eng = nc.sync if ko % 2 == 0 else nc.scalar
                        eng.dma_start(out=xts[:, ko, :], in_=xv[:, ko, :])

                    def qk_tile(i, qk, t):
                        ps = p1ps.tile([128, 512], fp32, tag="ps_qk",
                                       name="ps_qk")
                        w0 = i * 256 + qk * 128
                        for ko in range(KO):
                            nc.tensor.matmul(
                                ps,
                                wall_sb[:, ko, w0:w0 + 128],
                                xts[:, ko, t * 512:(t + 1) * 512],
                                start=(ko == 0),
                                stop=(ko == KO - 1),
                            )
                        # PSUM -> SBUF + per-partition bias
                        nc.scalar.activation(
                            qk2[i][:, qk, t * 512:(t + 1) * 512],
                            ps,
                            mybir.ActivationFunctionType.Identity,
                            bias=bqk_sb[:, i, qk:qk + 1],
                            scale=1.0,
                        )

                    def v_unit(kc):
                        psv = p1psv.tile([128, GD], fp32, tag="ps_v",
                                         name="ps_v")
                        for ko in range(KO):
                            nc.tensor.matmul(
                                psv,
                                xts[:, ko, kc * 128:(kc + 1) * 128],
                                wall_sb[:, ko, 512:768],
                                start=(ko == 0),
                                stop=False,
                            )
                        nc.tensor.matmul(psv, ones1, bv_sb, start=False,
                                         stop=True)
                        for h in range(HPC):
                            # VSCALE*V, cast to fp8, DoubleRow interleave slot
                            nc.vector.tensor_scalar_mul(
                                v65[:, kc // 2, h, kc % 2, 0:64],
                                psv[:, h * HD:(h + 1) * HD],
                                VSCALE,
                            )
                            nc.vector.tensor_copy(
                                v64b[:, kc, h, :], psv[:, h * HD:(h + 1) * HD]
                            )

                    # heads 0/1 need pair-0 QK: emit pair-0 upfront,
                    # interleave pair-1 QK + V production with S^T(0,1)
                    for qk in range(2):
                        for t in range(ST):
                            qk_tile(0, qk, t)
                    fill = []
                    for qk in range(2):
                        for t in range(ST):
                            fill.append(lambda qk=qk, t=t: qk_tile(1, qk, t))
                    for kc in range(SQ):
                        fill.append(lambda kc=kc: v_unit(kc))

                    e_h = [None] * HPC
                    e_h[0] = expstp.tile([128, SQ, S], fp8, tag="expst",
                                         name="expst0")
                    e_h[1] = expstp.tile([128, SQ, S], fp8, tag="expst",
                                         name="expst1")
                    emit_spair_interleaved(0, e_h[0], e_h[1], fill)

                # ---- phase 2b: U'/BV matmuls, outputs shipped raw ----
                with (
                    tc.tile_pool(name="expstp2", bufs=2) as expstp2,
                    tc.tile_pool(name="ubvpool", bufs=4, space="PSUM") as ubvpool,
                    tc.tile_pool(name="btring", bufs=4) as btring,
                    tc.tile_pool(name="asmpool", bufs=4) as asmpool,
                ):
                    def section_units(pairs):
                        """U'^T/BV^T work units for a list of (head, expst)
                        with BT prefetch one quarter ahead (ring bufs=4)."""
                        units = []
                        loaders = [(h, q, half) for h, _ in pairs
                                   for q in range(4) for half in range(2)]
                        bt_tiles = {}
                        li = [0]

                        def issue_until(n):
                            while li[0] < min(n, len(loaders)):
                                h, q, half = loaders[li[0]]
                                li[0] += 1
                                bt = btring.tile([128, 8, 512], bf16,
                                                 tag="bt", name="bt")
                                eng = (nc.sync if (q + half) % 2 == 0
                                       else nc.scalar)
                                if "btdma" not in skip:
                                    eng.dma_start(
                                        out=bt,
                                        in_=BT2[h, q][:, half * 8:
                                                      half * 8 + 8, :],
                                    )
                                bt_tiles[(h, q, half)] = bt

                        for hi, (h, e) in enumerate(pairs):
                            for q in range(4):
                                k = hi * 4 + q       # global quarter index
                                cell = {}

                                def start_q(cell=cell, k=k):
                                    # prefetch through NEXT quarter (ring=4)
                                    issue_until(2 * k + 4)
                                    cell["u"] = ubvpool.tile(
                                        [65, 512], fp32, tag="ub", name="u")
                                    cell["b"] = ubvpool.tile(
                                        [64, 512], fp32, tag="ub", name="bb")

                                units.append(start_q)

                                def mm_pair(kc2, cell=cell, h=h, q=q, e=e):
                                    st, sp = (kc2 == 0), (kc2 == KC2 - 1)
                                    if "umm" not in skip:
                                        nc.tensor.matmul(
                                            cell["u"],
                                            v65[:, kc2, h, :, 0:65],
                                            e[:, 2 * kc2:2 * kc2 + 2,
                                              q * 512:(q + 1) * 512],
                                            start=st, stop=sp, perf_mode=DR,
                                        )
                                    bt = bt_tiles[(h, q, kc2 // 4)]
                                    for j in range(2):
                                        if "bmm" in skip:
                                            break
                                        kc = 2 * kc2 + j
                                        nc.tensor.matmul(
                                            cell["b"],
                                            v64b[:, kc, h, :],
                                            bt[:, kc % 8, :],
                                            start=(kc == 0),
                                            stop=(kc == SQ - 1),
                                        )

                                for kc2 in range(KC2):
                                    units.append(
                                        lambda kc2=kc2, f=mm_pair: f(kc2))

                                def copies(cell=cell, h=h, q=q):
                                    if "asm" in skip:
                                        return
                                    usb = asmpool.tile([65, 512], bf16,
                                                       tag="usb", name="usb")
                                    bsb = asmpool.tile([64, 512], bf16,
                                                       tag="bsb", name="bsb")
                                    nc.vector.tensor_copy(usb, cell["u"])
                                    nc.vector.tensor_copy(bsb, cell["b"])
                                    nc.sync.dma_start(out=UO[h, q], in_=usb)
                                    nc.scalar.dma_start(out=BO[h, q], in_=bsb)

                                units.append(copies)
                        return units

                    e_h[2] = expstp2.tile([128, SQ, S], fp8, tag="expst",
                                          name="expst2")
                    e_h[3] = expstp2.tile([128, SQ, S], fp8, tag="expst",
                                          name="expst3")
                    fill2 = section_units([(0, e_h[0]), (1, e_h[1])])
                    emit_spair_interleaved(1, e_h[2], e_h[3], fill2)
                    for unit in section_units([(2, e_h[2]), (3, e_h[3])]):
                        unit()

    nc.finalize()
    return nc


def _prep_core_inputs(x, attn_B, W_attn, b_attn, core, BT_all=None):
    bi, g = core // 4, core % 4
    h0 = HPC * g
    xT = _to_bf16(np.ascontiguousarray(x[bi].T))             # [D, S]
    wqk = np.empty((D, 2, 2, 128), np.float32)
    bqk = np.empty((128, 2, 2), np.float32)
    for i in range(HPC // 2):
        for j in range(2):                                   # head within pair
            gh = h0 + 2 * i + j
            sl = slice(64 * j, 64 * j + 64)
            wqk[:, i, 0, sl] = W_attn[:, gh * 64:(gh + 1) * 64]
            wqk[:, i, 1, sl] = W_attn[:, D + gh * 64:D + (gh + 1) * 64]
            bqk[sl, i, 0] = b_attn[gh * 64:(gh + 1) * 64]
            bqk[sl, i, 1] = b_attn[D + gh * 64:D + (gh + 1) * 64]
    wv = W_attn[:, 2 * D + g * GD:2 * D + (g + 1) * GD]
    bv = b_attn[2 * D + g * GD:2 * D + (g + 1) * GD]
    # pack per-partition-contiguous: wall[p, ko, :512] = wqk, [512:] = wv
    wall = np.empty((128, KO, 768), np.float32)
    wall[:, :, 0:512] = wqk.reshape(KO, 128, 512).transpose(1, 0, 2)
    wall[:, :, 512:768] = wv.reshape(KO, 128, GD).transpose(1, 0, 2)
    if BT_all is None:
        BT_all = _prep_bt(attn_B)
    BT2 = np.ascontiguousarray(BT_all[bi, h0:h0 + HPC])
    return {
        "xT": xT, "wall": _to_bf16(wall), "bqk": bqk,
        "bv": _to_bf16(bv.reshape(1, GD)), "BT2": BT2,
    }


def _prep_bt(attn_B):
    """[b, h, sq, sk] f32 -> [b, h, q, p, kc, c] bf16 where
    BT2[b, h, q, p, kc, c] = attn_B[b, h, q*512+c, kc*128+p]."""
    a = _to_bf16(attn_B)                           # [b, h, 2048(sq), 2048(sk)]
    a = a.reshape(B, H, 4, 512, SQ, 128)           # [b, h, q, c, kc, p]
    return a.transpose(0, 1, 2, 5, 4, 3)           # [b, h, q, p, kc, c]


def _core_y(res_c):
    """Finalize one core's output: normalize the softmax term by the free
    denominators (row 64 of U'^T), add the bias term, transpose to [sq, hd].
    O(S*D) elementwise host work — negligible vs the O(S^2*D) device work."""
    UOc = res_c["UO"].astype(np.float32)        # [HPC, 4, 65, 512]
    BOc = res_c["BO"].astype(np.float32)        # [HPC, 4, 64, 512]
    yb = UOc[:, :, 0:64, :] / UOc[:, :, 64:65, :] + BOc
    # y[q*512+c, h*64+m] = yb[h, q, m, c]
    return yb.transpose(1, 3, 0, 2).reshape(S, GD)


def kernel(x, attn_B, W_attn, b_attn):
    global _CACHED_NC
    x = np.asarray(x, np.float32)
    attn_B = np.asarray(attn_B, np.float32)
    W_attn = np.asarray(W_attn, np.float32)
    b_attn = np.asarray(b_attn, np.float32)

    if _CACHED_NC is None:
        _CACHED_NC = build_nc()
    nc = _CACHED_NC

    BT_all = np.ascontiguousarray(_prep_bt(attn_B))
    in_maps = [
        _prep_core_inputs(x, attn_B, W_attn, b_attn, c, BT_all=BT_all)
        for c in range(NCORES)
    ]
    res = run_bass_kernel_spmd(nc, in_maps, core_ids=list(range(NCORES)))

    out = np.empty((B, S, D), np.float32)
    for c in range(NCORES):
        bi, g = c // 4, c % 4
        out[bi, :, g * GD:(g + 1) * GD] = _core_y(res.results[c])
    return out
